# revision 6
# baseline (speedup 1.0000x reference)
"""Trainium2 Bass kernel for nn_LlamaAttention_45749991637119.

Mathematical structure of the reference: K/V are a single shared head that
is broadcast across all 64 query heads, and attention is computed per token
position (no cross-token mixing).  scores[b,t,h,g] = q[b,t,h]·k[b,t] is
independent of g, so the softmax over g is exactly uniform (1/64) and
attn[b,t,h,:] == v[b,t,:] for every head h.  Therefore

    out = (hidden @ Wv.T) @ Wo_sum.T,   Wo_sum[i,d] = sum_h Wo[i, 64h+d]

and Wq/Wk/cos/sin never influence the output (verified to 5e-7 rel err
against the reference).

Device schedule per core (1024 tokens):

  stage A (v = Wv @ h^T): two 512-token SUPER-groups, col-tiled 2x —
    even k-chunks accumulate into PSUM partitions 0-63 (PE array cols
    0-63), odd chunks into partitions 64-127, CONCURRENTLY.  N=512 moving
    amortizes the ~150ns fixed LDWEIGHTS issue cost (the stage-A pacer).
    Produces a stacked [128, 512] psum: [vE; vO].
  stage B (out = v @ WoSum^T): four 256-token groups; the stacked vT
    (cast to bf16 by ACT) is a K=128 stationary, the moving operand is
    WoSum^T REPLICATED on both partition halves, so the matmul itself
    computes vE·woS + vO·woS = v·woS — full-array K=128 matmuls.
  drain: stage-B PSUM is copied to SBUF in 1024-col PAIRS split across
    Vector (row-block 0) and Scalar (row-block 1) engines — a single
    engine's ~1 elem/lane/cycle PSUM read rate would be the bottleneck.
  stores: one 256KB DMA per drained pair on gpsimd (SWDGE), so output
    DMA trickles out concurrently with everything else.

PE program order: [warmup dummies] A0 B0 B1 A1 B2 B3.  The ~20 dummy
matmuls run during the otherwise-idle DMA lead-in (~7-14us) purely to
hold the PE HAM clock-gate at 8/8 (2.4 GHz) before real work arrives.
ht loads are split in 8-chunk quarters so stage A tracks the DMA.

Sharding: data-parallel over tokens (B*T = 8192 -> 1024 per core).  All
inputs are packed on the host into ONE [128, 38912] bf16 tensor
(Wv^T chunks | WoSum^T x2 | hidden^T super-major).
"""

import numpy as np

import concourse.bass as bass
import concourse.mybir as mybir
from concourse.bass_utils import run_bass_kernel_spmd

N_CORES = 8
B, T, HID = 4, 2048, 4096
D = 64                      # v dim (head_dim)
TOKS = (B * T) // N_CORES   # 1024 tokens per core
P = 128                     # partitions
KC = HID // P               # 32 k-chunks
SG = 512                    # stage-A super-group tokens
NS = TOKS // SG             # 2 supers
TG = 256                    # stage-B token group
NG = TOKS // TG             # 4 groups
CD = 512                    # stage-B out-column tile
NCT = HID // CD             # 8 col tiles
NB = 6                      # stage-B psum ring (3 drain-pairs)
RB = TOKS // P              # 8 row-blocks (2 per group)
N_WARM = 20                 # PE warmup dummy matmuls

# packed input column offsets (bf16 elements per partition)
WV_COLS = KC * D            # 2048
WOS_COLS = HID              # 4096
HT_S_COLS = KC * SG         # 16384 per super
HT0 = WV_COLS + WOS_COLS    # 6144
PACK_COLS = HT0 + NS * HT_S_COLS  # 38912

COMPUTE_DTYPE = "bf16"
_CACHE = {}
LAST_RESULT = None

PE_ORDER = [("A", 0), ("B", 0), ("B", 1), ("A", 1), ("B", 2), ("B", 3)]


def _ticks():
    """Precompute semaphore tick tables for all engines."""
    a_tick, b_tick = {}, {}
    pe = 0
    for kind, g in PE_ORDER:
        if kind == "A":
            pe += 1
            a_tick[g] = pe
        else:
            for i in range(16):
                pe += 1
                b_tick[(g, i)] = pe

    # DVE: rb0 drain-pairs.  ACT: vT copies + rb1 drain-pairs.
    dve_prog, act_prog = [], []
    for kind, g in PE_ORDER:
        if kind == "A":
            act_prog.append(("vt", g))
        else:
            for pi in range(4):
                dve_prog.append(("dr", g, 0, pi))
                act_prog.append(("dr", g, 1, pi))

    vt_tick, pair_tick, pair_on_dve = {}, {}, {}
    t = 0
    for op in dve_prog:
        t += 1
        pair_tick[op[1:]] = t
        pair_on_dve[op[1:]] = True
    t = 0
    for op in act_prog:
        t += 1
        if op[0] == "vt":
            vt_tick[op[1]] = t
        else:
            pair_tick[op[1:]] = t
            pair_on_dve[op[1:]] = False
    return a_tick, b_tick, dve_prog, act_prog, vt_tick, pair_tick, pair_on_dve


def _build():
    dt_in = mybir.dt.bfloat16

    nc = bass.Bass()
    pack = nc.dram_tensor("pack", [P, PACK_COLS], dt_in, kind="ExternalInput")
    out = nc.dram_tensor("out", [TOKS, HID], dt_in, kind="ExternalOutput")

    (a_tick, b_tick, dve_prog, act_prog, vt_tick, pair_tick,
     pair_on_dve) = _ticks()

    # load pieces: wv, s0 q0..q2, woS2, s0 q3, s1 q0..q3 (x16 per piece)
    # stage-A quarter thresholds per super, indexed by c//8
    qt = {0: [32, 48, 64, 96], 1: [112, 128, 144, 160]}
    WOS_THRESH = 80

    with (
        nc.sbuf_tensor([P, PACK_COLS], dt_in) as mega,
        nc.sbuf_tensor([P, RB * HID], dt_in) as out_sb,
        nc.sbuf_tensor([P, TOKS], dt_in) as vT,
        nc.psum_tensor([P, SG]) as psv0,
        nc.psum_tensor([P, SG]) as psv1,
        nc.psum_tensor([P, NB * CD]) as psB,
        nc.semaphore() as s_load,
        nc.semaphore() as s_pe,
        nc.semaphore() as s_dve,
        nc.semaphore() as s_act,
        nc.semaphore() as s_store,
        nc.Block() as block,
    ):
        psv = [psv0, psv1]

        def wv_chunk(c):
            return mega[:, c * D:(c + 1) * D]

        def woS2(ct):
            return mega[:, WV_COLS + ct * CD:WV_COLS + (ct + 1) * CD]

        def ht(s, c):
            base = HT0 + s * HT_S_COLS + c * SG
            return mega[:, base:base + SG]

        @block.sync
        def _(sync):
            q = HT_S_COLS // 4  # 4096 cols = 8 chunks
            s0, s1 = HT0, HT0 + HT_S_COLS
            pieces = [(0, WV_COLS),
                      (s0, s0 + q), (s0 + q, s0 + 2 * q), (s0 + 2 * q, s0 + 3 * q),
                      (WV_COLS, HT0),
                      (s0 + 3 * q, s1),
                      (s1, s1 + q), (s1 + q, s1 + 2 * q),
                      (s1 + 2 * q, s1 + 3 * q), (s1 + 3 * q, s1 + 4 * q)]
            for lo, hi in pieces:
                sync.dma_start(out=mega[:, lo:hi], in_=pack[:, lo:hi]).then_inc(
                    s_load, 16
                )

        @block.tensor
        def _(tensor):
            waited = {}

            def wait(sem, name, val):
                if waited.get(name, 0) < val:
                    waited[name] = val
                    tensor.wait_ge(sem, val)

            # Warmup: keep the PE busy during the DMA lead-in so HAM
            # un-throttles to 2.4 GHz before real matmuls arrive.  Reads
            # uninitialized SBUF (harmless); results overwritten by the
            # first real start=True matmul into each psB slot.
            for w in range(N_WARM):
                tensor.matmul(
                    psB[:, (w % NB) * CD:(w % NB) * CD + TG],
                    mega[:, 0:P],
                    mega[:, 0:TG],
                    start=True, stop=True,
                )

            for kind, g in PE_ORDER:
                if kind == "A":
                    for c in range(KC):
                        if c % 8 == 0:
                            wait(s_load, "load", qt[g][c // 8])
                        half = c % 2
                        mm = tensor.matmul(
                            psv[g][half * D:(half + 1) * D, :],
                            wv_chunk(c),
                            ht(g, c),
                            start=(c < 2),
                            stop=(c >= KC - 2),
                        )
                        if c == KC - 1:
                            mm.then_inc(s_pe, 1)
                else:
                    wait(s_act, "act", vt_tick[g // 2])
                    if g == 0:
                        wait(s_load, "load", WOS_THRESH)
                    for i in range(16):
                        j = g * 16 + i
                        if j >= NB:
                            gp, ip = divmod(j - NB, 16)
                            key = (gp, ip // 8, (ip % 8) // 2)
                            if pair_on_dve[key]:
                                wait(s_dve, "dve", pair_tick[key])
                            else:
                                wait(s_act, "act", pair_tick[key])
                        slot = j % NB
                        rb, ct = divmod(i, 8)
                        tensor.matmul(
                            psB[:, slot * CD:(slot + 1) * CD],
                            vT[:, (g * 2 + rb) * P:(g * 2 + rb + 1) * P],
                            woS2(ct),
                            start=True, stop=True,
                        ).then_inc(s_pe, 1)

        @block.vector
        def _(vector):
            for _, g, rb, pi in dve_prog:
                i = rb * 8 + 2 * pi
                j = g * 16 + i
                vector.wait_ge(s_pe, b_tick[(g, i + 1)])
                slot = j % NB
                r = g * 2 + rb
                vector.tensor_copy(
                    out=out_sb[:, r * HID + 2 * pi * CD:r * HID + (2 * pi + 2) * CD],
                    in_=psB[:, slot * CD:(slot + 2) * CD],
                ).then_inc(s_dve, 1)

        @block.scalar
        def _(scalar):
            for op in act_prog:
                if op[0] == "vt":
                    s = op[1]
                    scalar.wait_ge(s_pe, a_tick[s])
                    scalar.activation(
                        out=vT[:, s * SG:(s + 1) * SG],
                        in_=psv[s][:, :],
                        func=mybir.ActivationFunctionType.Copy,
                    ).then_inc(s_act, 1)
                else:
                    _, g, rb, pi = op
                    i = rb * 8 + 2 * pi
                    j = g * 16 + i
                    scalar.wait_ge(s_pe, b_tick[(g, i + 1)])
                    slot = j % NB
                    r = g * 2 + rb
                    scalar.activation(
                        out=out_sb[:, r * HID + 2 * pi * CD:
                                   r * HID + (2 * pi + 2) * CD],
                        in_=psB[:, slot * CD:(slot + 2) * CD],
                        func=mybir.ActivationFunctionType.Copy,
                    ).then_inc(s_act, 1)

        @block.gpsimd
        def _(gpsimd):
            n_store = 0
            for _, g in [x for x in PE_ORDER if x[0] == "B"]:
                for rb in range(2):
                    r = g * 2 + rb
                    for pi in range(4):
                        key = (g, rb, pi)
                        if pair_on_dve[key]:
                            gpsimd.wait_ge(s_dve, pair_tick[key])
                        else:
                            gpsimd.wait_ge(s_act, pair_tick[key])
                        c0 = 2 * pi * CD
                        gpsimd.dma_start(
                            out=out[r * P:(r + 1) * P, c0:c0 + 2 * CD],
                            in_=out_sb[:, r * HID + c0:r * HID + c0 + 2 * CD],
                        ).then_inc(s_store, 16)
                        n_store += 1
            gpsimd.wait_ge(s_store, 16 * n_store)
    return nc


def kernel(hidden_states, cos, sin, Wq, Wk, Wv, Wo):
    global LAST_RESULT
    import ml_dtypes
    np_bf16 = ml_dtypes.bfloat16

    if "nc" not in _CACHE:
        _CACHE["nc"] = _build()
    nc = _CACHE["nc"]

    hidden_states = np.asarray(hidden_states, dtype=np.float32)
    Wv = np.asarray(Wv, dtype=np.float32)
    Wo = np.asarray(Wo, dtype=np.float32)

    flat = hidden_states.reshape(B * T, HID)
    # Wv^T chunks: pack[p, c*64+d] = Wv[d, c*128+p]
    wv_part = np.ascontiguousarray(
        Wv.reshape(D, KC, P).transpose(2, 1, 0).reshape(P, KC * D)
    ).astype(np_bf16)
    # Wo_sum^T replicated on both partition halves: pack[p, j] = woS[p%64, j]
    woS = Wo.reshape(HID, HID // D, D).sum(axis=1, dtype=np.float32).T  # [64, 4096]
    woS2_part = np.concatenate([woS, woS], axis=0).astype(np_bf16)      # [128, 4096]

    in_maps = []
    for jc in range(N_CORES):
        blk = flat[jc * TOKS:(jc + 1) * TOKS, :]          # [1024, 4096]
        # ht super-major: pack[p, s*16384 + c*512 + t] = blk[s*512+t, c*128+p]
        ht_part = np.ascontiguousarray(
            blk.reshape(NS, SG, KC, P).transpose(3, 0, 2, 1).reshape(P, NS * HT_S_COLS)
        ).astype(np_bf16)
        packed = np.concatenate([wv_part, woS2_part, ht_part], axis=1)
        in_maps.append({"pack": np.ascontiguousarray(packed)})

    LAST_RESULT = run_bass_kernel_spmd(nc, in_maps, core_ids=list(range(N_CORES)))
    outs = [np.asarray(LAST_RESULT.results[jc]["out"]).astype(np.float32)
            for jc in range(N_CORES)]
    return np.concatenate(outs, axis=0).reshape(B, T, HID)


# revision 7
# speedup vs baseline: 1.0879x; 1.0879x over previous
"""Trainium2 Bass kernel for nn_LlamaAttention_45749991637119.

Mathematical structure of the reference: K/V are a single shared head that
is broadcast across all 64 query heads, and attention is computed per token
position (no cross-token mixing).  scores[b,t,h,g] = q[b,t,h]·k[b,t] is
independent of g, so the softmax over g is exactly uniform (1/64) and
attn[b,t,h,:] == v[b,t,:] for every head h.  Therefore

    out = (hidden @ Wv.T) @ Wo_sum.T,   Wo_sum[i,d] = sum_h Wo[i, 64h+d]

and Wq/Wk/cos/sin never influence the output (verified to 5e-7 rel err
against the reference).

Device schedule per core (1024 tokens):

  stage A (v = Wv @ h^T): two 512-token SUPER-groups, col-tiled 2x —
    even k-chunks accumulate into PSUM partitions 0-63 (PE array cols
    0-63), odd chunks into partitions 64-127, CONCURRENTLY.  N=512 moving
    amortizes the ~150ns fixed LDWEIGHTS issue cost (the stage-A pacer).
    Produces a stacked [128, 512] psum: [vE; vO].
  stage B (out = v @ WoSum^T): four 256-token groups; the stacked vT
    (cast to bf16 by ACT) is a K=128 stationary, the moving operand is
    WoSum^T REPLICATED on both partition halves, so the matmul itself
    computes vE·woS + vO·woS = v·woS — full-array K=128 matmuls.
  drain: stage-B PSUM is copied to SBUF in 1024-col PAIRS split across
    Vector (row-block 0) and Scalar (row-block 1) engines — a single
    engine's ~1 elem/lane/cycle PSUM read rate would be the bottleneck.
  stores: one 256KB DMA per drained pair on gpsimd (SWDGE), so output
    DMA trickles out concurrently with everything else.

PE program order: [warmup dummies] A0 B0 B1 A1 B2 B3.  The ~20 dummy
matmuls run during the otherwise-idle DMA lead-in (~7-14us) purely to
hold the PE HAM clock-gate at 8/8 (2.4 GHz) before real work arrives.
ht loads are split in 8-chunk quarters so stage A tracks the DMA.

Sharding: data-parallel over tokens (B*T = 8192 -> 1024 per core).  All
inputs are packed on the host into ONE [128, 38912] bf16 tensor
(Wv^T chunks | WoSum^T x2 | hidden^T super-major).
"""

import numpy as np

import concourse.bass as bass
import concourse.mybir as mybir
from concourse.bass_utils import run_bass_kernel_spmd

N_CORES = 8
B, T, HID = 4, 2048, 4096
D = 64                      # v dim (head_dim)
TOKS = (B * T) // N_CORES   # 1024 tokens per core
P = 128                     # partitions
KC = HID // P               # 32 k-chunks
SG = 512                    # stage-A super-group tokens
NS = TOKS // SG             # 2 supers
TG = 256                    # stage-B token group
NG = TOKS // TG             # 4 groups
CD = 512                    # stage-B out-column tile
NCT = HID // CD             # 8 col tiles
NB = 6                      # stage-B psum ring (3 drain-pairs)
RB = TOKS // P              # 8 row-blocks (2 per group)
N_WARM = 20                 # PE warmup dummy matmuls

# packed input column offsets (bf16 elements per partition)
WV_COLS = KC * D            # 2048
WOS_COLS = HID              # 4096
HT_S_COLS = KC * SG         # 16384 per super
HT0 = WV_COLS + WOS_COLS    # 6144
PACK_COLS = HT0 + NS * HT_S_COLS  # 38912

COMPUTE_DTYPE = "bf16"
_CACHE = {}
LAST_RESULT = None

PE_ORDER = [("A", 0), ("B", 0), ("B", 1), ("A", 1), ("B", 2), ("B", 3)]


def _ticks():
    """Precompute semaphore tick tables for all engines."""
    a_tick, b_tick = {}, {}
    pe = 0
    for kind, g in PE_ORDER:
        if kind == "A":
            pe += 1
            a_tick[g] = pe
        else:
            for i in range(16):
                pe += 1
                b_tick[(g, i)] = pe

    # DVE: rb0 drain-pairs.  ACT: vT copies + rb1 drain-pairs.
    dve_prog, act_prog = [], []
    for kind, g in PE_ORDER:
        if kind == "A":
            act_prog.append(("vt", g))
        else:
            for pi in range(4):
                dve_prog.append(("dr", g, 0, pi))
                act_prog.append(("dr", g, 1, pi))

    vt_tick, pair_tick, pair_on_dve = {}, {}, {}
    t = 0
    for op in dve_prog:
        t += 1
        pair_tick[op[1:]] = t
        pair_on_dve[op[1:]] = True
    t = 0
    for op in act_prog:
        t += 1
        if op[0] == "vt":
            vt_tick[op[1]] = t
        else:
            pair_tick[op[1:]] = t
            pair_on_dve[op[1:]] = False
    return a_tick, b_tick, dve_prog, act_prog, vt_tick, pair_tick, pair_on_dve


def _build():
    dt_in = mybir.dt.bfloat16

    nc = bass.Bass()
    pack = nc.dram_tensor("pack", [P, PACK_COLS], dt_in, kind="ExternalInput")
    out = nc.dram_tensor("out", [TOKS, HID], dt_in, kind="ExternalOutput")

    (a_tick, b_tick, dve_prog, act_prog, vt_tick, pair_tick,
     pair_on_dve) = _ticks()

    # load pieces: wv, s0 q0-q2, woS2a, s0 q3, woS2b, s1 q0-q3 (x16 each)
    # stage-A quarter thresholds per super, indexed by c//8
    qt = {0: [32, 48, 64, 96], 1: [128, 144, 160, 176]}
    WOS_A_THRESH = 80   # woS2 cols 0-2047 (ct 0-3)
    WOS_B_THRESH = 112  # woS2 cols 2048-4095 (ct 4-7)

    with (
        nc.sbuf_tensor([P, PACK_COLS], dt_in) as mega,
        nc.sbuf_tensor([P, RB * HID], dt_in) as out_sb,
        nc.sbuf_tensor([P, TOKS], dt_in) as vT,
        nc.psum_tensor([P, SG]) as psv0,
        nc.psum_tensor([P, SG]) as psv1,
        nc.psum_tensor([P, NB * CD]) as psB,
        nc.semaphore() as s_load,
        nc.semaphore() as s_pe,
        nc.semaphore() as s_dve,
        nc.semaphore() as s_act,
        nc.semaphore() as s_store,
        nc.Block() as block,
    ):
        psv = [psv0, psv1]

        def wv_chunk(c):
            return mega[:, c * D:(c + 1) * D]

        def woS2(ct):
            return mega[:, WV_COLS + ct * CD:WV_COLS + (ct + 1) * CD]

        def ht(s, c):
            base = HT0 + s * HT_S_COLS + c * SG
            return mega[:, base:base + SG]

        @block.sync
        def _(sync):
            q = HT_S_COLS // 4  # 4096 cols = 8 chunks
            s0, s1 = HT0, HT0 + HT_S_COLS
            wmid = WV_COLS + WOS_COLS // 2
            pieces = [(0, WV_COLS),
                      (s0, s0 + q), (s0 + q, s0 + 2 * q), (s0 + 2 * q, s0 + 3 * q),
                      (WV_COLS, wmid),
                      (s0 + 3 * q, s1),
                      (wmid, HT0),
                      (s1, s1 + q), (s1 + q, s1 + 2 * q),
                      (s1 + 2 * q, s1 + 3 * q), (s1 + 3 * q, s1 + 4 * q)]
            for lo, hi in pieces:
                sync.dma_start(out=mega[:, lo:hi], in_=pack[:, lo:hi]).then_inc(
                    s_load, 16
                )

        @block.tensor
        def _(tensor):
            waited = {}

            def wait(sem, name, val):
                if waited.get(name, 0) < val:
                    waited[name] = val
                    tensor.wait_ge(sem, val)

            # Warmup: keep the PE busy during the DMA lead-in so HAM
            # un-throttles to 2.4 GHz before real matmuls arrive.  Reads
            # uninitialized SBUF (harmless); results overwritten by the
            # first real start=True matmul into each psB slot.
            for w in range(N_WARM):
                tensor.matmul(
                    psB[:, (w % NB) * CD:(w % NB) * CD + TG],
                    mega[:, 0:P],
                    mega[:, 0:TG],
                    start=True, stop=True,
                )

            for kind, g in PE_ORDER:
                if kind == "A":
                    for c in range(KC):
                        if c % 8 == 0:
                            wait(s_load, "load", qt[g][c // 8])
                        half = c % 2
                        mm = tensor.matmul(
                            psv[g][half * D:(half + 1) * D, :],
                            wv_chunk(c),
                            ht(g, c),
                            start=(c < 2),
                            stop=(c >= KC - 2),
                        )
                        if c == KC - 1:
                            mm.then_inc(s_pe, 1)
                else:
                    wait(s_act, "act", vt_tick[g // 2])
                    if g == 0:
                        wait(s_load, "load", WOS_A_THRESH)
                    for i in range(16):
                        if g == 0 and i == 4:
                            wait(s_load, "load", WOS_B_THRESH)
                        j = g * 16 + i
                        if j >= NB:
                            gp, ip = divmod(j - NB, 16)
                            key = (gp, ip // 8, (ip % 8) // 2)
                            if pair_on_dve[key]:
                                wait(s_dve, "dve", pair_tick[key])
                            else:
                                wait(s_act, "act", pair_tick[key])
                        slot = j % NB
                        rb, ct = divmod(i, 8)
                        tensor.matmul(
                            psB[:, slot * CD:(slot + 1) * CD],
                            vT[:, (g * 2 + rb) * P:(g * 2 + rb + 1) * P],
                            woS2(ct),
                            start=True, stop=True,
                        ).then_inc(s_pe, 1)

        @block.vector
        def _(vector):
            for _, g, rb, pi in dve_prog:
                i = rb * 8 + 2 * pi
                j = g * 16 + i
                vector.wait_ge(s_pe, b_tick[(g, i + 1)])
                slot = j % NB
                r = g * 2 + rb
                vector.tensor_copy(
                    out=out_sb[:, r * HID + 2 * pi * CD:r * HID + (2 * pi + 2) * CD],
                    in_=psB[:, slot * CD:(slot + 2) * CD],
                ).then_inc(s_dve, 1)

        @block.scalar
        def _(scalar):
            # preload the activation table set (Copy) during the DMA
            # lead-in so the one-time ~1.5us ACT_TABLE_LOAD is off the
            # critical path; reads uninitialized psum, result unused.
            scalar.activation(
                out=vT[0:1, 0:8], in_=psv0[0:1, 0:8],
                func=mybir.ActivationFunctionType.Copy,
            )
            for op in act_prog:
                if op[0] == "vt":
                    s = op[1]
                    scalar.wait_ge(s_pe, a_tick[s])
                    scalar.activation(
                        out=vT[:, s * SG:(s + 1) * SG],
                        in_=psv[s][:, :],
                        func=mybir.ActivationFunctionType.Copy,
                    ).then_inc(s_act, 1)
                else:
                    _, g, rb, pi = op
                    i = rb * 8 + 2 * pi
                    j = g * 16 + i
                    scalar.wait_ge(s_pe, b_tick[(g, i + 1)])
                    slot = j % NB
                    r = g * 2 + rb
                    scalar.activation(
                        out=out_sb[:, r * HID + 2 * pi * CD:
                                   r * HID + (2 * pi + 2) * CD],
                        in_=psB[:, slot * CD:(slot + 2) * CD],
                        func=mybir.ActivationFunctionType.Copy,
                    ).then_inc(s_act, 1)

        @block.gpsimd
        def _(gpsimd):
            n_store = 0
            for _, g in [x for x in PE_ORDER if x[0] == "B"]:
                for rb in range(2):
                    r = g * 2 + rb
                    for pi in range(4):
                        key = (g, rb, pi)
                        if pair_on_dve[key]:
                            gpsimd.wait_ge(s_dve, pair_tick[key])
                        else:
                            gpsimd.wait_ge(s_act, pair_tick[key])
                        c0 = 2 * pi * CD
                        gpsimd.dma_start(
                            out=out[r * P:(r + 1) * P, c0:c0 + 2 * CD],
                            in_=out_sb[:, r * HID + c0:r * HID + c0 + 2 * CD],
                        ).then_inc(s_store, 16)
                        n_store += 1
            gpsimd.wait_ge(s_store, 16 * n_store)
    return nc


def kernel(hidden_states, cos, sin, Wq, Wk, Wv, Wo):
    global LAST_RESULT
    import ml_dtypes
    np_bf16 = ml_dtypes.bfloat16

    if "nc" not in _CACHE:
        _CACHE["nc"] = _build()
    nc = _CACHE["nc"]

    hidden_states = np.asarray(hidden_states, dtype=np.float32)
    Wv = np.asarray(Wv, dtype=np.float32)
    Wo = np.asarray(Wo, dtype=np.float32)

    flat = hidden_states.reshape(B * T, HID)
    # Wv^T chunks: pack[p, c*64+d] = Wv[d, c*128+p]
    wv_part = np.ascontiguousarray(
        Wv.reshape(D, KC, P).transpose(2, 1, 0).reshape(P, KC * D)
    ).astype(np_bf16)
    # Wo_sum^T replicated on both partition halves: pack[p, j] = woS[p%64, j]
    woS = Wo.reshape(HID, HID // D, D).sum(axis=1, dtype=np.float32).T  # [64, 4096]
    woS2_part = np.concatenate([woS, woS], axis=0).astype(np_bf16)      # [128, 4096]

    in_maps = []
    for jc in range(N_CORES):
        blk = flat[jc * TOKS:(jc + 1) * TOKS, :]          # [1024, 4096]
        # ht super-major: pack[p, s*16384 + c*512 + t] = blk[s*512+t, c*128+p]
        ht_part = np.ascontiguousarray(
            blk.reshape(NS, SG, KC, P).transpose(3, 0, 2, 1).reshape(P, NS * HT_S_COLS)
        ).astype(np_bf16)
        packed = np.concatenate([wv_part, woS2_part, ht_part], axis=1)
        in_maps.append({"pack": np.ascontiguousarray(packed)})

    LAST_RESULT = run_bass_kernel_spmd(nc, in_maps, core_ids=list(range(N_CORES)))
    outs = [np.asarray(LAST_RESULT.results[jc]["out"]).astype(np.float32)
            for jc in range(N_CORES)]
    return np.concatenate(outs, axis=0).reshape(B, T, HID)


# revision 8
# speedup vs baseline: 1.1417x; 1.0495x over previous
"""Trainium2 Bass kernel for nn_LlamaAttention_45749991637119.

Mathematical structure of the reference: K/V are a single shared head that
is broadcast across all 64 query heads, and attention is computed per token
position (no cross-token mixing).  scores[b,t,h,g] = q[b,t,h]·k[b,t] is
independent of g, so the softmax over g is exactly uniform (1/64) and
attn[b,t,h,:] == v[b,t,:] for every head h.  Therefore

    out = (hidden @ Wv.T) @ Wo_sum.T,   Wo_sum[i,d] = sum_h Wo[i, 64h+d]

and Wq/Wk/cos/sin never influence the output (verified to 5e-7 rel err
against the reference).

Device schedule per core (1024 tokens):

  stage A (v = Wv @ h^T): two 512-token SUPER-groups, col-tiled 2x —
    even k-chunks accumulate into PSUM partitions 0-63 (PE array cols
    0-63), odd chunks into partitions 64-127, CONCURRENTLY.  N=512 moving
    amortizes the ~150ns fixed LDWEIGHTS issue cost (the stage-A pacer).
    Produces a stacked [128, 512] psum: [vE; vO].
  stage B (out = v @ WoSum^T): four 256-token groups; the stacked vT
    (cast to bf16 by ACT) is a K=128 stationary, the moving operand is
    WoSum^T REPLICATED on both partition halves, so the matmul itself
    computes vE·woS + vO·woS = v·woS — full-array K=128 matmuls.
  drain: stage-B PSUM is copied to SBUF in 1024-col PAIRS split across
    Vector (row-block 0) and Scalar (row-block 1) engines — a single
    engine's ~1 elem/lane/cycle PSUM read rate would be the bottleneck.
  stores: one 256KB DMA per drained pair on gpsimd (SWDGE), so output
    DMA trickles out concurrently with everything else.

PE program order: [warmup dummies] A0 B0 B1 A1 B2 B3.  The ~20 dummy
matmuls run during the otherwise-idle DMA lead-in (~7-14us) purely to
hold the PE HAM clock-gate at 8/8 (2.4 GHz) before real work arrives.
ht loads are split in 8-chunk quarters so stage A tracks the DMA.

Sharding: data-parallel over tokens (B*T = 8192 -> 1024 per core).  All
inputs are packed on the host into ONE [128, 38912] bf16 tensor
(Wv^T chunks | WoSum^T x2 | hidden^T super-major).
"""

import numpy as np

import concourse.bass as bass
import concourse.mybir as mybir
from concourse.bass_utils import run_bass_kernel_spmd

N_CORES = 8
B, T, HID = 4, 2048, 4096
D = 64                      # v dim (head_dim)
TOKS = (B * T) // N_CORES   # 1024 tokens per core
P = 128                     # partitions
KC = HID // P               # 32 k-chunks
SG = 512                    # stage-A super-group tokens
NS = TOKS // SG             # 2 supers
TG = 256                    # stage-B token group
NG = TOKS // TG             # 4 groups
CD = 512                    # stage-B out-column tile
NCT = HID // CD             # 8 col tiles
NB = 6                      # stage-B psum ring (3 drain-pairs)
RB = TOKS // P              # 8 row-blocks (2 per group)
N_WARM = 24                 # PE warmup dummy matmuls

# packed input column offsets (bf16 elements per partition)
WV_COLS = KC * D            # 2048
WOS_COLS = HID              # 4096
HT_S_COLS = KC * SG         # 16384 per super
HT0 = WV_COLS + WOS_COLS    # 6144
PACK_COLS = HT0 + NS * HT_S_COLS  # 38912

COMPUTE_DTYPE = "bf16"
_CACHE = {}
LAST_RESULT = None

PE_ORDER = [("A", 0), ("B", 0), ("B", 1), ("A", 1), ("B", 2), ("B", 3)]


def _ticks():
    """Precompute semaphore tick tables for all engines."""
    a_tick, b_tick = {}, {}
    pe = 0
    for kind, g in PE_ORDER:
        if kind == "A":
            pe += 1
            a_tick[g] = pe
        else:
            for i in range(16):
                pe += 1
                b_tick[(g, i)] = pe

    # DVE: rb0 drain-pairs.  ACT: vT copies + rb1 drain-pairs.
    dve_prog, act_prog = [], []
    for kind, g in PE_ORDER:
        if kind == "A":
            act_prog.append(("vt", g))
        else:
            for pi in range(4):
                dve_prog.append(("dr", g, 0, pi))
                act_prog.append(("dr", g, 1, pi))

    vt_tick, pair_tick, pair_on_dve = {}, {}, {}
    t = 0
    for op in dve_prog:
        t += 1
        pair_tick[op[1:]] = t
        pair_on_dve[op[1:]] = True
    t = 0
    for op in act_prog:
        t += 1
        if op[0] == "vt":
            vt_tick[op[1]] = t
        else:
            pair_tick[op[1:]] = t
            pair_on_dve[op[1:]] = False
    return a_tick, b_tick, dve_prog, act_prog, vt_tick, pair_tick, pair_on_dve


def _build():
    dt_in = mybir.dt.bfloat16

    nc = bass.Bass()
    pack = nc.dram_tensor("pack", [P, PACK_COLS], dt_in, kind="ExternalInput")
    out = nc.dram_tensor("out", [TOKS, HID], dt_in, kind="ExternalOutput")

    (a_tick, b_tick, dve_prog, act_prog, vt_tick, pair_tick,
     pair_on_dve) = _ticks()

    # load pieces: wv, s0 q0-q2, woS2a, s0 q3, woS2b, s1 q0-q3 (x16 each)
    # stage-A quarter thresholds per super, indexed by c//8
    qt = {0: [32, 48, 64, 96], 1: [128, 144, 160, 176]}
    WOS_A_THRESH = 80   # woS2 cols 0-2047 (ct 0-3)
    WOS_B_THRESH = 112  # woS2 cols 2048-4095 (ct 4-7)

    with (
        nc.sbuf_tensor([P, PACK_COLS], dt_in) as mega,
        nc.sbuf_tensor([P, RB * HID], dt_in) as out_sb,
        nc.sbuf_tensor([P, TOKS], dt_in) as vT,
        nc.psum_tensor([P, SG]) as psv0,
        nc.psum_tensor([P, SG]) as psv1,
        nc.psum_tensor([P, NB * CD]) as psB,
        nc.semaphore() as s_load,
        nc.semaphore() as s_pe,
        nc.semaphore() as s_dve,
        nc.semaphore() as s_act,
        nc.semaphore() as s_store,
        nc.Block() as block,
    ):
        psv = [psv0, psv1]

        def wv_chunk(c):
            return mega[:, c * D:(c + 1) * D]

        def woS2(ct):
            return mega[:, WV_COLS + ct * CD:WV_COLS + (ct + 1) * CD]

        def ht(s, c):
            base = HT0 + s * HT_S_COLS + c * SG
            return mega[:, base:base + SG]

        @block.sync
        def _(sync):
            q = HT_S_COLS // 4  # 4096 cols = 8 chunks
            s0, s1 = HT0, HT0 + HT_S_COLS
            wmid = WV_COLS + WOS_COLS // 2
            pieces = [(0, WV_COLS),
                      (s0, s0 + q), (s0 + q, s0 + 2 * q), (s0 + 2 * q, s0 + 3 * q),
                      (WV_COLS, wmid),
                      (s0 + 3 * q, s1),
                      (wmid, HT0),
                      (s1, s1 + q), (s1 + q, s1 + 2 * q),
                      (s1 + 2 * q, s1 + 3 * q), (s1 + 3 * q, s1 + 4 * q)]
            for lo, hi in pieces:
                sync.dma_start(out=mega[:, lo:hi], in_=pack[:, lo:hi]).then_inc(
                    s_load, 16
                )
            # stores: HWDGE on the (otherwise idle) sync engine — avoids
            # the multi-us SWDGE ring-drain postamble gpsimd stores pay.
            n_store = 0
            for _, g in [x for x in PE_ORDER if x[0] == "B"]:
                for rb in range(2):
                    r = g * 2 + rb
                    for h in range(2):
                        key = (g, rb, 2 * h + 1)
                        if pair_on_dve[key]:
                            sync.wait_ge(s_dve, pair_tick[key])
                        else:
                            sync.wait_ge(s_act, pair_tick[key])
                        c0 = h * (HID // 2)
                        sync.dma_start(
                            out=out[r * P:(r + 1) * P, c0:c0 + HID // 2],
                            in_=out_sb[:, r * HID + c0:r * HID + c0 + HID // 2],
                        ).then_inc(s_store, 16)
                        n_store += 1
            sync.wait_ge(s_store, 16 * n_store)

        @block.tensor
        def _(tensor):
            waited = {}

            def wait(sem, name, val):
                if waited.get(name, 0) < val:
                    waited[name] = val
                    tensor.wait_ge(sem, val)

            # Warmup: keep the PE busy during the DMA lead-in so HAM
            # un-throttles to 2.4 GHz before real matmuls arrive.  Reads
            # uninitialized SBUF (harmless); results overwritten by the
            # first real start=True matmul into each psB slot.
            for w in range(N_WARM):
                tensor.matmul(
                    psB[:, (w % NB) * CD:(w % NB) * CD + TG],
                    mega[:, 0:P],
                    mega[:, 0:TG],
                    start=True, stop=True,
                )

            def mini_warm(n=2):
                # tiny dummy matmuls emitted just before a wait that may
                # stall on DMA: keeps the PE HAM activity window busy so
                # the clock stays at 8/8 through stage-A's paced stalls.
                for w in range(n):
                    tensor.matmul(
                        psB[:, 0:P], mega[:, 0:P], mega[:, 0:P],
                        start=True, stop=True, skip_group_check=True,
                    )

            for kind, g in PE_ORDER:
                if kind == "A":
                    for c in range(KC):
                        if c % 8 == 0:
                            if g == 0:
                                mini_warm()
                            wait(s_load, "load", qt[g][c // 8])
                        half = c % 2
                        mm = tensor.matmul(
                            psv[g][half * D:(half + 1) * D, :],
                            wv_chunk(c),
                            ht(g, c),
                            start=(c < 2),
                            stop=(c >= KC - 2),
                        )
                        if c == KC - 1:
                            mm.then_inc(s_pe, 1)
                else:
                    if g == 0:
                        mini_warm()
                    wait(s_act, "act", vt_tick[g // 2])
                    if g == 0:
                        wait(s_load, "load", WOS_A_THRESH)
                    for i in range(16):
                        if g == 0 and i == 4:
                            wait(s_load, "load", WOS_B_THRESH)
                        j = g * 16 + i
                        if j >= NB:
                            gp, ip = divmod(j - NB, 16)
                            key = (gp, ip // 8, (ip % 8) // 2)
                            if pair_on_dve[key]:
                                wait(s_dve, "dve", pair_tick[key])
                            else:
                                wait(s_act, "act", pair_tick[key])
                        slot = j % NB
                        rb, ct = divmod(i, 8)
                        tensor.matmul(
                            psB[:, slot * CD:(slot + 1) * CD],
                            vT[:, (g * 2 + rb) * P:(g * 2 + rb + 1) * P],
                            woS2(ct),
                            start=True, stop=True,
                        ).then_inc(s_pe, 1)

        @block.vector
        def _(vector):
            for _, g, rb, pi in dve_prog:
                i = rb * 8 + 2 * pi
                j = g * 16 + i
                vector.wait_ge(s_pe, b_tick[(g, i + 1)])
                slot = j % NB
                r = g * 2 + rb
                vector.tensor_copy(
                    out=out_sb[:, r * HID + 2 * pi * CD:r * HID + (2 * pi + 2) * CD],
                    in_=psB[:, slot * CD:(slot + 2) * CD],
                ).then_inc(s_dve, 1)

        @block.scalar
        def _(scalar):
            # preload the activation table set (Copy) during the DMA
            # lead-in so the one-time ~1.5us ACT_TABLE_LOAD is off the
            # critical path; reads uninitialized psum, result unused.
            scalar.activation(
                out=vT[0:1, 0:8], in_=psv0[0:1, 0:8],
                func=mybir.ActivationFunctionType.Copy,
            )
            for op in act_prog:
                if op[0] == "vt":
                    s = op[1]
                    scalar.wait_ge(s_pe, a_tick[s])
                    scalar.activation(
                        out=vT[:, s * SG:(s + 1) * SG],
                        in_=psv[s][:, :],
                        func=mybir.ActivationFunctionType.Copy,
                    ).then_inc(s_act, 1)
                else:
                    _, g, rb, pi = op
                    i = rb * 8 + 2 * pi
                    j = g * 16 + i
                    scalar.wait_ge(s_pe, b_tick[(g, i + 1)])
                    slot = j % NB
                    r = g * 2 + rb
                    scalar.activation(
                        out=out_sb[:, r * HID + 2 * pi * CD:
                                   r * HID + (2 * pi + 2) * CD],
                        in_=psB[:, slot * CD:(slot + 2) * CD],
                        func=mybir.ActivationFunctionType.Copy,
                    ).then_inc(s_act, 1)

    return nc


def kernel(hidden_states, cos, sin, Wq, Wk, Wv, Wo):
    global LAST_RESULT
    import ml_dtypes
    np_bf16 = ml_dtypes.bfloat16

    if "nc" not in _CACHE:
        _CACHE["nc"] = _build()
    nc = _CACHE["nc"]

    hidden_states = np.asarray(hidden_states, dtype=np.float32)
    Wv = np.asarray(Wv, dtype=np.float32)
    Wo = np.asarray(Wo, dtype=np.float32)

    flat = hidden_states.reshape(B * T, HID)
    # Wv^T chunks: pack[p, c*64+d] = Wv[d, c*128+p]
    wv_part = np.ascontiguousarray(
        Wv.reshape(D, KC, P).transpose(2, 1, 0).reshape(P, KC * D)
    ).astype(np_bf16)
    # Wo_sum^T replicated on both partition halves: pack[p, j] = woS[p%64, j]
    woS = Wo.reshape(HID, HID // D, D).sum(axis=1, dtype=np.float32).T  # [64, 4096]
    woS2_part = np.concatenate([woS, woS], axis=0).astype(np_bf16)      # [128, 4096]

    in_maps = []
    for jc in range(N_CORES):
        blk = flat[jc * TOKS:(jc + 1) * TOKS, :]          # [1024, 4096]
        # ht super-major: pack[p, s*16384 + c*512 + t] = blk[s*512+t, c*128+p]
        ht_part = np.ascontiguousarray(
            blk.reshape(NS, SG, KC, P).transpose(3, 0, 2, 1).reshape(P, NS * HT_S_COLS)
        ).astype(np_bf16)
        packed = np.concatenate([wv_part, woS2_part, ht_part], axis=1)
        in_maps.append({"pack": np.ascontiguousarray(packed)})

    LAST_RESULT = run_bass_kernel_spmd(nc, in_maps, core_ids=list(range(N_CORES)))
    outs = [np.asarray(LAST_RESULT.results[jc]["out"]).astype(np.float32)
            for jc in range(N_CORES)]
    return np.concatenate(outs, axis=0).reshape(B, T, HID)
